# revision 24
# baseline (speedup 1.0000x reference)
"""Trainium2 Bass kernel for YOLO-style detection decode (nms_detection).

Computes, for input `output` (B=8, H=80, W=80, A*85=255):
  per (b, cell, anchor):  xy = (sigmoid(txy) + grid_off) * stride
                          wh = exp(twh) * anchor
                          bbox = [xy - wh/2, xy + wh/2]
                          p_c = sigmoid(cls_c) * sigmoid(obj)
  out (B, C*hw*A, 6) rows = [cid, score, x1, y1, x2, y2] where
  cid = c if p_c > 0.01 else -1, score = p_c if p_c > 0.01 else 0.

Sharding: pure data parallel over batch, one batch element per NeuronCore.

v2 design (vs the f32 baseline):
  - output is written bf16 on device (rel err ~4e-3, gate is 2e-2) and
    upcast to f32 on the host -> output DMA traffic halves to 18.4 MB/core
  - host repacks the input: only the 243 channels that need a sigmoid
    (cls+obj) stream per supertile as [part, t, 243] contiguous blocks;
    the 12 xy/wh channels ride the consts blob and are decoded for the
    whole core in ~8 large ops before the supertile loop
  - supertiles of 1024 cells (8 subtiles of 128)
  - class scores staged bf16 -> PE transposes run at 1 cyc/row and land
    bf16 in PSUM, which lets the threshold mask run in the DVE 2x mode
  - cid moves to ScalarE as one activation op per supertile:
    Copy(mask * (c+1)) + (-1) with a per-partition scale AP
  - bbox is broadcast to the 80 class partitions with one-hot selector
    matmuls (bf16, K=12) and the PSUM->output copies are spread across
    ScalarE/VectorE/GpSimd
"""

import sys
import os
from contextlib import ExitStack

if "/opt/trn_rl_repo" not in sys.path:
    sys.path.insert(0, "/opt/trn_rl_repo")

import numpy as np

NUM_CLASSES = 80
NUM_ANCHOR = 3
NUM_PRED = 85
HW_CELLS = 6400
N_TILES = HW_CELLS // 128  # 50
THRESH = 0.01
N_CORES = 8
ROW = 6 * NUM_ANCHOR  # 18
CIN = NUM_ANCHOR * NUM_CLASSES + NUM_ANCHOR  # 243 streamed channels
SW = NUM_ANCHOR * NUM_CLASSES + 16 + 12  # 268: [cls 240 | pad 16 | bb 12]
SUPER = 1024  # cells per supertile

# consts blob (f32) column offsets
OFF_OFFS = 0                      # [t, a, k] grid offsets * stride   (300)
OFF_HANCH = OFF_OFFS + N_TILES * 6  # anchor/2 per (a, k)             (6)
OFF_CP1 = OFF_HANCH + 6           # c+1 per class partition           (1)
OFF_M1 = OFF_CP1 + 1              # constant -1.0                     (1)
OFF_XY = OFF_M1 + 1               # xy channels [t, a, k]             (300)
OFF_WH = OFF_XY + N_TILES * 6     # wh channels [t, a, k]             (300)
CF_F = OFF_WH + N_TILES * 6       # 907
# bf16 blob column offsets
OFF_IDENT = 0                     # identity 128x128
OFF_SEL = OFF_IDENT + 128         # one-hot selectors [12, 12*80] @ rows 96:108
CB_F = OFF_SEL + 12 * NUM_CLASSES  # 1088

_CACHE = {}
LAST_RESULT = None  # BassKernelResults of the most recent kernel() call


def _build(stride_f: float, n_cells: int = HW_CELLS):
    import concourse.bass as bass  # noqa: F401
    import concourse.bacc as bacc
    import concourse.tile as tile
    from concourse import mybir

    f32 = mybir.dt.float32
    bf16 = mybir.dt.bfloat16
    AF = mybir.ActivationFunctionType
    OP = mybir.AluOpType

    C = NUM_CLASSES
    A = NUM_ANCHOR

    nc = bacc.Bacc("TRN2", target_bir_lowering=False, debug=False)
    x_d = nc.declare_dram_parameter("x", [128, N_TILES * CIN], bf16, isOutput=False)
    cf_d = nc.declare_dram_parameter("cf", [128, CF_F], f32, isOutput=False)
    cb_d = nc.declare_dram_parameter("cb", [128, CB_F], bf16, isOutput=False)
    out_d = nc.declare_dram_parameter("out", [C, n_cells * ROW], bf16, isOutput=True)

    # channels replicated via SBUF->SBUF DMA tree (anchors 0,1); anchor 2
    # stays on PE broadcast matmuls
    NDMA = 8

    st_sizes = []
    left = n_cells
    while left > 0:
        take = min(SUPER, left)
        assert take % 128 == 0
        st_sizes.append(take)
        left -= take

    with ExitStack() as ctx:
        tc = ctx.enter_context(tile.TileContext(nc))
        cpool = ctx.enter_context(tc.tile_pool(name="const", bufs=1))
        wpool = ctx.enter_context(tc.tile_pool(name="work", bufs=1))
        in_pool = ctx.enter_context(tc.tile_pool(name="inp", bufs=3))
        sig_pool = ctx.enter_context(tc.tile_pool(name="sig", bufs=2))
        s_pool = ctx.enter_context(tc.tile_pool(name="scls", bufs=2))
        m_pool = ctx.enter_context(tc.tile_pool(name="mask", bufs=2))
        bb_pool = ctx.enter_context(tc.tile_pool(name="bbt", bufs=2))
        rep_pool = ctx.enter_context(tc.tile_pool(name="rep", bufs=2))
        o_pool = ctx.enter_context(tc.tile_pool(name="outt", bufs=2))
        p_pool = ctx.enter_context(tc.tile_pool(name="ppsum", bufs=3, space="PSUM"))
        q_pool = ctx.enter_context(tc.tile_pool(name="qpsum", bufs=2, space="PSUM"))

        # ---- constants ----
        cf = cpool.tile([128, CF_F], f32, tag="cf")
        nc.scalar.dma_start(out=cf[:, :], in_=cf_d[:, :])
        cb = cpool.tile([128, CB_F], bf16, tag="cb")
        nc.scalar.dma_start(out=cb[:, :], in_=cb_d[:, :])

        offs = cf[:, OFF_OFFS:OFF_HANCH]
        hanch = cf[:, OFF_HANCH:OFF_CP1]
        cp1 = cf[:, OFF_CP1 : OFF_CP1 + 1]
        m1 = cf[:, OFF_M1 : OFF_M1 + 1]
        xyc = cf[:, OFF_XY:OFF_WH]
        whc = cf[:, OFF_WH:CF_F]
        ident = cb[:, OFF_IDENT:OFF_SEL]
        sel = cb[:, OFF_SEL:CB_F]

        # ---- warm-up: every engine observes both const DMAs once ----
        warm = cpool.tile([128, 8], f32, tag="warm")
        nc.vector.tensor_copy(warm[0:1, 0:1], cf[0:1, 0:1])
        nc.vector.tensor_copy(warm[0:1, 1:2], cb[0:1, 0:1])
        nc.scalar.copy(warm[0:1, 2:3], cf[0:1, 0:1])
        nc.scalar.copy(warm[0:1, 3:4], cb[0:1, 0:1])
        nc.gpsimd.tensor_copy(warm[0:1, 4:5], cf[0:1, 0:1])
        nc.gpsimd.tensor_copy(warm[0:1, 5:6], cb[0:1, 0:1])
        wq = p_pool.tile([128, 1024], bf16, tag="P")
        nc.tensor.transpose(wq[0:128, 0:128], ident, ident)

        # ---- whole-core bbox precompute (cell-major, [128, t, a, k]) ----
        TAK = N_TILES * 6  # 300
        sigxy = wpool.tile([128, TAK], f32, tag="sigxy")
        swh = wpool.tile([128, TAK], f32, tag="swh")
        snw = wpool.tile([128, TAK], f32, tag="snw")
        rec = wpool.tile([128, TAK], f32, tag="rec")
        t1 = wpool.tile([128, TAK], f32, tag="t1")
        halfwh = wpool.tile([128, TAK], f32, tag="halfwh")
        xypx = wpool.tile([128, TAK], f32, tag="xypx")
        bb_all = wpool.tile([128, N_TILES * 12], bf16, tag="bb_all")

        nc.scalar.activation(sigxy[:, :], xyc, AF.Sigmoid)
        nc.scalar.activation(swh[:, :], whc, AF.Sigmoid)
        nc.scalar.activation(snw[:, :], whc, AF.Sigmoid, scale=-1.0)
        nc.vector.reciprocal(rec[:, :], snw[:, :])
        # halfwh = sig(wh) * (anchor/2) / sig(-wh)
        nc.vector.tensor_tensor(
            t1[:, :].rearrange("p (t j) -> p t j", j=6),
            swh[:, :].rearrange("p (t j) -> p t j", j=6),
            hanch.rearrange("p (o j) -> p o j", o=1).to_broadcast([128, N_TILES, 6]),
            OP.mult,
        )
        nc.vector.tensor_tensor(halfwh[:, :], t1[:, :], rec[:, :], OP.mult)
        # xy = sigmoid(xy)*stride + off*stride
        nc.vector.scalar_tensor_tensor(
            xypx[:, :], in0=sigxy[:, :], scalar=stride_f, in1=offs, op0=OP.mult, op1=OP.add
        )
        bb_v = bb_all[:, :].rearrange("p (t a k) -> p t a k", a=A, k=4)
        xy_v = xypx[:, :].rearrange("p (t a k) -> p t a k", a=A, k=2)
        hw_v = halfwh[:, :].rearrange("p (t a k) -> p t a k", a=A, k=2)
        nc.vector.tensor_tensor(bb_v[:, :, :, 0:2], xy_v, hw_v, OP.subtract)
        nc.vector.tensor_tensor(bb_v[:, :, :, 2:4], xy_v, hw_v, OP.add)

        c0 = 0
        for st, ncell in enumerate(st_sizes):
            ns = ncell // 128
            t0 = c0 // 128
            nh = max(1, ncell // 512)  # 512-cell halves
            hsz = ncell // nh

            # ---- load input supertile [128, ns*243] (contiguous, bf16) ----
            in_t = in_pool.tile([128, ns * CIN], bf16, tag="in")
            nc.scalar.dma_start(out=in_t[:, :], in_=x_d[:, t0 * CIN : (t0 + ns) * CIN])

            # ---- sigmoid over cls+obj ----
            sig = sig_pool.tile([128, ns * CIN], bf16, tag="sig")
            nc.scalar.activation(sig[:, :], in_t[:, :], AF.Sigmoid)
            sig_v = sig[:, :].rearrange("p (s c) -> p s c", c=CIN)

            # ---- class scores (bf16) + bbox staging columns ----
            S = s_pool.tile([128, ns * SW], bf16, tag="S")
            S_v = S[:, :].rearrange("p (s w) -> p s w", w=SW)
            nc.gpsimd.tensor_tensor(
                S_v[:, :, 0 : A * C].rearrange("p s (a c) -> p s a c", c=C),
                sig_v[:, :, 0 : A * C].rearrange("p s (a c) -> p s a c", c=C),
                sig_v[:, :, A * C : A * C + A]
                .rearrange("p s (a o) -> p s a o", o=1)
                .to_broadcast([128, ns, A, C]),
                OP.mult,
            )
            nc.vector.tensor_copy(
                S_v[:, :, A * C + 16 : SW],
                bb_all[:, t0 * 12 : (t0 + ns) * 12].rearrange(
                    "p (s j) -> p s j", j=12
                ),
            )

            # ---- transposes: per (subtile, anchor); anchor 2 carries bbox ----
            P = [None] * A
            for a in (2, 0, 1):
                pw = C + 28 if a == 2 else C
                P[a] = p_pool.tile([128, ncell], bf16, tag="P", name=f"P{a}")
                for s in range(ns):
                    nc.tensor.transpose(
                        P[a][0:pw, s * 128 : (s + 1) * 128],
                        S_v[:, s, a * C : a * C + pw],
                        ident,
                    )
                if a == 2:
                    bbt = bb_pool.tile([128, ncell], bf16, tag="bbt")
                    nc.vector.tensor_copy(bbt[96:108, :], P[a][96:108, :])
                    # seed + tree-replicate channels 0..NDMA-1 (anchors 0,1)
                    # to all 80 class partitions: rep[c, ch, i]
                    rep = rep_pool.tile([80, NDMA * ncell], bf16, tag="rep")
                    rep_v = rep[:, :].rearrange("c (j i) -> c j i", j=NDMA)
                    for ch in range(NDMA):
                        nc.sync.dma_start(
                            out=rep_v[0:1, ch, :], in_=bbt[96 + ch : 97 + ch, :]
                        )
                    lo = 1
                    while lo < 80:
                        n = min(lo, 80 - lo)
                        nc.sync.dma_start(
                            out=rep[lo : lo + n, :], in_=rep[0:n, :]
                        )
                        lo += n

            # ---- output supertile ----
            outt = o_pool.tile([C, ncell * ROW], bf16, tag="outt")
            ov = outt[:, :].rearrange("c (i a f) -> c a f i", a=A, f=6)

            # ---- mask + cid/score planes (all contiguous bf16 writes) ----
            mask3 = m_pool.tile([C, A * ncell], bf16, tag="mask3")
            m_v = mask3[:, :].rearrange("c (a i) -> c a i", a=A)
            for a in range(A):
                nc.vector.tensor_scalar(
                    m_v[:, a, :], P[a][0:C, :], THRESH, None, OP.is_gt
                )
            # cs planes: [e in {cid, score}][a][i], contiguous per plane
            cs = m_pool.tile([C, 2 * A * ncell], bf16, tag="cs")
            cs_v = cs[:, :].rearrange("c (e a i) -> c e a i", e=2, a=A)
            nc.gpsimd.tensor_scalar(
                cs_v[:, 0, :, :], m_v[:, :, :], cp1[0:C, :], -1.0, OP.mult, OP.add
            )
            for a in range(A):
                nc.vector.tensor_tensor(
                    cs_v[:, 1, a, :], P[a][0:C, :], m_v[:, a, :], OP.mult
                )
            # pair-gather (cid,score) -> out cols (0,1): strided read, 4B writes
            for a in range(A):
                gsrc = cs_v[:, :, a, :].rearrange("c e i -> c i e")
                gdst = ov[:, a, 0:2, :].rearrange("c e i -> c i e")
                if a == 0:
                    nc.scalar.copy(gdst, gsrc)
                else:
                    nc.vector.tensor_copy(gdst, gsrc)

            # ---- bbox anchors 0,1: pair mini-gathers from the DMA-replicated
            # planar tile (all-bf16, hits the DVE 2x mode) ----
            gi = 0
            for a in range(NDMA // 4):
                for p in range(2):
                    j0 = a * 4 + 2 * p
                    gsrc = rep_v[:, j0 : j0 + 2, :].rearrange("c e i -> c i e")
                    gdst = ov[:, a, 2 + 2 * p : 4 + 2 * p, :].rearrange(
                        "c e i -> c i e"
                    )
                    if gi % 2 == 0:
                        nc.scalar.copy(gdst, gsrc)
                    else:
                        nc.vector.tensor_copy(gdst, gsrc)
                    gi += 1

            # ---- bbox anchor 2: planar pair matmuls + pair mini-gathers ----
            for a in range(NDMA // 4, A):
                for p in range(2):
                    j0 = a * 4 + 2 * p
                    for h in range(nh):
                        q = q_pool.tile([C, 2 * hsz], f32, tag="q")
                        for e in range(2):
                            nc.tensor.matmul(
                                q[:, e * hsz : (e + 1) * hsz],
                                lhsT=sel[96:108, (j0 + e) * C : (j0 + e + 1) * C],
                                rhs=bbt[96:108, h * hsz : (h + 1) * hsz],
                                start=True,
                                stop=True,
                                tile_position=(96, 0),
                            )
                        gsrc = q[:, :].rearrange("c (e i) -> c i e", e=2)
                        gdst = ov[
                            :, a, 2 + 2 * p : 4 + 2 * p, h * hsz : (h + 1) * hsz
                        ].rearrange("c e i -> c i e")
                        if gi % 2 == 0:
                            nc.scalar.copy(gdst, gsrc)
                        else:
                            nc.vector.tensor_copy(gdst, gsrc)
                        gi += 1

            # ---- store (alternate between the two HWDGE rings) ----
            oeng = nc.sync if st % 2 == 0 else nc.scalar
            oeng.dma_start(
                out=out_d[:, c0 * ROW : (c0 + ncell) * ROW], in_=outt[:, :]
            )
            c0 += ncell

    nc.finalize()
    return nc


def make_consts(anchor, offset, stride_f, x_xyw):
    """cf blob: [offs*stride | anchor/2 | c+1 | xy | wh], all [128, F] f32.

    x_xyw: [128, t, a, 4] xy+wh channels of this core's input (cell-major,
    partition p holds cell t*128+p). Returned per-core.
    """
    off = np.asarray(offset, dtype=np.float32).reshape(-1, 2)[:HW_CELLS] * stride_f
    offs = np.tile(off, (1, 3)).reshape(N_TILES, 128, 6)
    offs = np.transpose(offs, (1, 0, 2)).reshape(128, N_TILES * 6)
    a2 = np.asarray(anchor, dtype=np.float32).reshape(NUM_ANCHOR, 2)
    hanch = np.tile((a2 / 2.0).reshape(1, 6), (128, 1)).astype(np.float32)
    cp1 = np.broadcast_to(
        np.arange(1, 129, dtype=np.float32).reshape(128, 1), (128, 1)
    )
    m1 = np.full((128, 1), -1.0, dtype=np.float32)
    xy = x_xyw[:, :, :, 0:2].reshape(128, N_TILES * 6)
    wh = x_xyw[:, :, :, 2:4].reshape(128, N_TILES * 6)
    blob = np.concatenate([offs, hanch, cp1, m1, xy, wh], axis=1).astype(np.float32)
    assert blob.shape[1] == CF_F
    return np.ascontiguousarray(blob)


def make_cb16():
    import ml_dtypes

    ident = np.eye(128, dtype=np.float32)
    sel = np.zeros((128, 12 * NUM_CLASSES), dtype=np.float32)
    for j in range(12):
        sel[96 + j, j * NUM_CLASSES : (j + 1) * NUM_CLASSES] = 1.0
    blob = np.concatenate([ident, sel], axis=1)
    assert blob.shape[1] == CB_F
    return np.ascontiguousarray(blob.astype(ml_dtypes.bfloat16))


def _host_prep(output, anchor, offset, stride):
    stride_f = float(stride)
    B = output.shape[0]
    x = np.asarray(output, dtype=np.float32).reshape(B, N_TILES, 128, NUM_ANCHOR, NUM_PRED)
    # streamed channels: [cls a0 | cls a1 | cls a2 | obj a0..a2], cell-major
    import ml_dtypes

    cls = np.transpose(x[..., 5:], (0, 2, 1, 3, 4)).reshape(B, 128, N_TILES, 240)
    obj = np.transpose(x[..., 4:5], (0, 2, 1, 3, 4)).reshape(B, 128, N_TILES, 3)
    x2 = np.concatenate([cls, obj], axis=3).reshape(B, 128, N_TILES * CIN)
    x2 = np.ascontiguousarray(x2.astype(ml_dtypes.bfloat16))
    # xy+wh channels, [B, 128, t, a, 4]
    xyw = np.ascontiguousarray(np.transpose(x[..., 0:4], (0, 2, 1, 3, 4)))
    cb = make_cb16()
    cfs = [make_consts(anchor, offset, stride_f, xyw[b]) for b in range(B)]
    return stride_f, x2, cfs, cb


def kernel(output, anchor, offset, stride):
    from concourse.bass_utils import run_bass_kernel_spmd

    stride_f, x2, cfs, cb = _host_prep(output, anchor, offset, stride)
    key = ("nc", stride_f)
    if key not in _CACHE:
        _CACHE[key] = _build(stride_f)
    nc = _CACHE[key]

    in_maps = [{"x": x2[b], "cf": cfs[b], "cb": cb} for b in range(N_CORES)]
    res = run_bass_kernel_spmd(
        nc,
        in_maps,
        list(range(N_CORES)),
        tmpdir=os.environ.get("KERNEL_TRACE_DIR") or None,
    )
    global LAST_RESULT
    LAST_RESULT = res
    outs = [
        r["out"].astype(np.float32).reshape(NUM_CLASSES * HW_CELLS * NUM_ANCHOR, 6)
        for r in res.results
    ]
    return np.stack(outs, axis=0)


if __name__ == "__main__":
    rng = np.random.default_rng(0)
    out = rng.standard_normal((8, 80, 80, 255)).astype(np.float32)
    anchor = rng.uniform(10.0, 120.0, (1, 1, 3, 2)).astype(np.float32)
    gy, gx = np.meshgrid(np.arange(80, dtype=np.float32), np.arange(80, dtype=np.float32), indexing="ij")
    offset = np.stack([gx, gy], axis=-1).reshape(1, 80, 80, 1, 2)
    r = kernel(out, anchor, offset, 8)
    print(r.shape, r.dtype)
